# revision 1
# baseline (speedup 1.0000x reference)
"""Trainium2 Bass kernel for nn_AttentionHead (B=4, T=4096, D=1024, H=64).

Sharding: 8 cores; core i handles (batch b = i//2, T-half = i%2): computes
attention output for its 2048 queries. K/V are computed per-core over the
full 4096 keys (weights tiny/replicated; key order is permutation-invariant
under softmax, so own-half-first ordering per core is fine).

Per-core dataflow (big matmuls in float32r = full-rate fp32 on the PE;
walrus requires fp32r operands to be produced *rounded*, so every matmul
input comes from a DVE copy or ACT activation with fp32r output dtype):
  - x inputs are typed float32r so the 256 PE transposes to x^T run in
    transpose mode at 1.5 cyc/row (walrus accepts external-input f32r);
    the PE pass itself rounds x to f32r precision.
  - Projections use host-concatenated stationary weights so one M=128
    pass computes two heads at once (M does not affect matmul time):
    first T-half runs [Wk|Wq] (k -> kTp partitions 0:64, q -> 64:128,
    then DMA-duplicated down), second T-half runs [Wv|Wk] so k lands
    directly at partitions 64:128 of kTp. relu+bias on ACT writes the
    persistent fp32r tiles in place; scores then row-pack two
    64-contraction matmuls (row groups 0/64) per PSUM tile.
  - v_T PE-transposed to V natural [t,64]; column 64 = ones so attn@V also
    accumulates the softmax denominator.
  - scores s_T[k,q]: two k-tiles row-packed (contraction=64, row groups
    0/64) into one PSUM [128,1024] tile; exp on ACT with scale=1/8 (no max
    subtraction: scores are O(1) by construction). Two q-blocks are
    interleaved per k-pair so PE work hides the ACT exp chain.
  - attn@V: V'[128,65] stationary x exp[128,512] accumulated over 32
    k-tiles into PSUM [65,512]; row 64 = denominator. PE-transpose back,
    reciprocal*scale on DVE, DMA out.

Tensors are split at group granularity (kTp/Vg/qTb) so the Tile scheduler
can overlap the projection stage with attention as dependencies resolve.
"""

import os
import numpy as np

B, T, D, H = 4, 4096, 1024, 64
P = 128
NB = 512            # free-dim block size
TQ = T // 2         # queries per core
NCORES = 8

_cache = {}


def _build(use_f32r=True):
    import concourse.bass as bass
    import concourse.tile as tile
    from concourse import bacc, mybir
    from concourse.masks import make_identity

    f32 = mybir.dt.float32
    f32r = mybir.dt.float32r
    AF = mybir.ActivationFunctionType

    mmdt = f32r if use_f32r else f32

    nc = bacc.Bacc("TRN2", target_bir_lowering=False, debug=False)

    xa = nc.dram_tensor("xa", [TQ, D], mmdt, kind="ExternalInput").ap()
    xb = nc.dram_tensor("xb", [TQ, D], mmdt, kind="ExternalInput").ap()
    wkq = nc.dram_tensor("wkq", [D, P], f32, kind="ExternalInput").ap()
    wvk = nc.dram_tensor("wvk", [D, P], f32, kind="ExternalInput").ap()
    wvt = nc.dram_tensor("wvt", [D, H], f32, kind="ExternalInput").ap()
    bkq = nc.dram_tensor("bkq", [P, 1], f32, kind="ExternalInput").ap()
    bvk = nc.dram_tensor("bvk", [P, 1], f32, kind="ExternalInput").ap()
    bv = nc.dram_tensor("bv", [H, 1], f32, kind="ExternalInput").ap()
    out = nc.dram_tensor("o", [TQ, H], f32, kind="ExternalOutput").ap()

    NG = T // NB          # 8 K/V t-groups of 512
    NGH = NG // 2         # 4 groups per T-half
    NQB = TQ // NB        # 4 q-blocks of 512
    NKP = T // P // 2     # 16 k-tile pairs
    NC = D // P           # 8 d-chunks
    NJ = NB // P          # 4 t-subtiles per group

    with tile.TileContext(nc) as tc:
        with (
            tc.tile_pool(name="const", bufs=1) as constp,
            tc.tile_pool(name="persist", bufs=1) as persist,
            tc.tile_pool(name="xg", bufs=6) as xgp,
            tc.tile_pool(name="xt", bufs=8) as xtp,
            tc.tile_pool(name="kvsb", bufs=2) as kvp,
            tc.tile_pool(name="big_ps", bufs=2, space="PSUM") as bigp,
            tc.tile_pool(name="proj_ps", bufs=2, space="PSUM") as proj_psp,
            tc.tile_pool(name="o_ps", bufs=2, space="PSUM") as o_psp,
            tc.tile_pool(name="esb", bufs=4) as esbp,
            tc.tile_pool(name="osb", bufs=2) as osbp,
            tc.tile_pool(name="outp", bufs=2) as outp,
        ):
            ident = constp.tile([P, P], f32)
            make_identity(nc, ident)
            ident_r = constp.tile([P, P], mmdt)
            nc.vector.tensor_copy(ident_r, ident)
            wkq_ld = constp.tile([P, NC, P], f32)
            wvk_ld = constp.tile([P, NC, P], f32)
            wv_ld = constp.tile([P, NC, H], f32)
            wkq_sb = constp.tile([P, NC, P], mmdt)
            wvk_sb = constp.tile([P, NC, P], mmdt)
            wv_sb = constp.tile([P, NC, H], mmdt)
            bkq_sb = constp.tile([P, 1], f32)
            bvk_sb = constp.tile([P, 1], f32)
            bv_sb = constp.tile([H, 1], f32)

            def load_consts():
                # emitted after the first x-block DMA so the transposes (the
                # first PE work) aren't stuck behind the weight loads; DVE
                # copies round fp32 -> fp32r (walrus requires rounded inputs)
                nc.sync.dma_start(wkq_ld, wkq.rearrange("(c p) h -> p c h", p=P))
                nc.sync.dma_start(wvk_ld, wvk.rearrange("(c p) h -> p c h", p=P))
                nc.sync.dma_start(wv_ld, wvt.rearrange("(c p) h -> p c h", p=P))
                nc.vector.tensor_copy(wkq_sb, wkq_ld)
                nc.vector.tensor_copy(wvk_sb, wvk_ld)
                nc.vector.tensor_copy(wv_sb, wv_ld)
                nc.sync.dma_start(bkq_sb, bkq)
                nc.sync.dma_start(bvk_sb, bvk)
                nc.sync.dma_start(bv_sb, bv)

            # persistent attention operands, split per group for overlap
            kTp = [persist.tile([P, NJ, P], mmdt, name=f"kTp{j}") for j in range(NGH)]
            qTb = [persist.tile([P, NB], mmdt, name=f"qTb{j}") for j in range(NQB)]
            Vg = [persist.tile([P, NJ, H + 1], mmdt, name=f"Vg{g}") for g in range(NG)]
            onesc = constp.tile([P, NJ, 1], f32)
            nc.gpsimd.memset(onesc, 1.0)
            for g in range(NG):
                nc.vector.tensor_copy(Vg[g][:, :, H : H + 1], onesc)

            # ---------------- projections ----------------
            def do_group(g, after_dma=None):
                half2 = g >= NGH          # second T-half (keys 2048..4095)
                src = xb if half2 else xa
                j = g % NGH
                r0 = j * NB
                # two half-loads so transposes start after the first lands
                xga = xgp.tile([P, NJ // 2, D], mmdt, tag="xg")
                xgb = xgp.tile([P, NJ // 2, D], mmdt, tag="xg")
                nc.sync.dma_start(
                    xga,
                    src[r0 : r0 + NB // 2, :].rearrange("(j p) d -> p j d", p=P),
                )
                nc.sync.dma_start(
                    xgb,
                    src[r0 + NB // 2 : r0 + NB, :].rearrange("(j p) d -> p j d", p=P),
                )
                if after_dma is not None:
                    after_dma()

                def xg(jj):
                    return (xga if jj < NJ // 2 else xgb)[:, jj % (NJ // 2), :]
                # transpose to x^T: c-pairs staged through one [128,1024] bank-pair
                xts = []
                for cp in range(NC // 2):
                    pt = bigp.tile([P, 2 * NB], mmdt, tag="big")
                    for ci in range(2):
                        c = 2 * cp + ci
                        for jj in range(NJ):
                            nc.tensor.transpose(
                                pt[:, ci * NB + jj * P : ci * NB + (jj + 1) * P],
                                xg(jj)[:, c * P : (c + 1) * P],
                                ident_r,
                            )
                    xt = xtp.tile([P, 2, NB], mmdt)
                    nc.vector.tensor_copy(xt, pt.rearrange("p (c n) -> p c n", c=2))
                    xts.append(xt)

                # Projections with concatenated stationary weights: one
                # M=128 pass computes two heads at once (M does not affect
                # matmul time). First half: [Wk|Wq] -> k at partitions 0:64
                # (kTp half A) and q at 64:128 (row-packed scores' B operand).
                # Second half: [Wv|Wk] -> k lands directly at partitions
                # 64:128 of kTp (no partition-shift DMA needed).
                w2 = wvk_sb if half2 else wkq_sb
                kq_ps = proj_psp.tile([P, NB], f32, tag="proj", name="kq_ps")
                for c in range(NC):
                    nc.tensor.matmul(
                        kq_ps,
                        w2[:, c, :],
                        xts[c // 2][:, c % 2, :],
                        start=(c == 0),
                        stop=(c == NC - 1),
                    )
                b2 = bvk_sb if half2 else bkq_sb
                if not half2:
                    # k rows 0:64 -> kTp half A; q rows 64:128 -> qTb
                    nc.scalar.activation(
                        kTp[j][0:H, :, :].rearrange("h j t -> h (j t)"),
                        kq_ps[0:H, :], AF.Relu, bias=b2[0:H, 0:1],
                    )
                    nc.scalar.activation(
                        qTb[j][H:P, :], kq_ps[H:P, :], AF.Relu,
                        bias=b2[H:P, 0:1],
                    )
                    nc.sync.dma_start(qTb[j][0:H, :], qTb[j][H:P, :])
                    # V in its own pass
                    v_ps = proj_psp.tile([H, NB], f32, tag="proj", name="v_ps")
                    for c in range(NC):
                        nc.tensor.matmul(
                            v_ps,
                            wv_sb[:, c, :],
                            xts[c // 2][:, c % 2, :],
                            start=(c == 0),
                            stop=(c == NC - 1),
                        )
                    v_sb = kvp.tile([H, NB], mmdt, tag="kvsb", name="v_sb")
                    nc.scalar.activation(v_sb, v_ps, AF.Relu, bias=bv_sb[:, 0:1])
                else:
                    # v rows 0:64; k rows 64:128 -> kTp half B directly
                    v_sb = kvp.tile([H, NB], mmdt, tag="kvsb", name="v_sb")
                    nc.scalar.activation(
                        v_sb, kq_ps[0:H, :], AF.Relu, bias=b2[0:H, 0:1]
                    )
                    nc.scalar.activation(
                        kTp[j][H:P, :, :].rearrange("h j t -> h (j t)"),
                        kq_ps[H:P, :], AF.Relu, bias=b2[H:P, 0:1],
                    )
                vt_ps = proj_psp.tile([P, NJ, H], mmdt, tag="proj", name="vt_ps")
                for jj in range(NJ):
                    nc.tensor.transpose(
                        vt_ps[:, jj, :],
                        v_sb[:, jj * P : (jj + 1) * P],
                        ident_r[0:H, 0:H],
                    )
                nc.vector.tensor_copy(Vg[g][:, :, 0:H], vt_ps)

            for j in range(NGH):
                do_group(j, after_dma=load_consts if j == 0 else None)
                do_group(j + NGH)

            # ---------------- attention ----------------
            scale = float(1.0 / np.sqrt(H))
            for qbp in range(NQB // 2):
                qbs = (2 * qbp, 2 * qbp + 1)
                o_ps = {
                    qb: o_psp.tile([H + 1, NB], f32, name=f"o_ps{qb}", tag="o_ps")
                    for qb in qbs
                }
                for p in range(NKP):
                    jg, i = p // NJ, p % NJ
                    e2 = {}
                    for qb in qbs:
                        s2 = bigp.tile([P, 2 * NB], f32, tag="big")
                        nc.tensor.matmul(
                            s2[:, 0:NB],
                            kTp[jg][0:H, i, :],
                            qTb[qb][0:H, :],
                            start=True,
                            stop=True,
                            tile_position=(0, 0),
                        )
                        nc.tensor.matmul(
                            s2[:, NB : 2 * NB],
                            kTp[jg][H:P, i, :],
                            qTb[qb][H:P, :],
                            start=True,
                            stop=True,
                            tile_position=(H, 0),
                        )
                        e = esbp.tile([P, 2 * NB], mmdt)
                        nc.scalar.activation(e, s2, AF.Exp, scale=scale)
                        e2[qb] = e
                    for qb in qbs:
                        nc.tensor.matmul(
                            o_ps[qb],
                            Vg[jg][:, i, :],
                            e2[qb][:, 0:NB],
                            start=(p == 0),
                            stop=False,
                        )
                        nc.tensor.matmul(
                            o_ps[qb],
                            Vg[NGH + jg][:, i, :],
                            e2[qb][:, NB : 2 * NB],
                            start=False,
                            stop=(p == NKP - 1),
                        )
                # normalize and store
                for qb in qbs:
                    o_sb = osbp.tile([H + 1, NB], f32)
                    nc.vector.tensor_copy(o_sb, o_ps[qb])
                    o4 = outp.tile([P, NJ, H], f32)
                    for jj in range(NJ):
                        ot = bigp.tile([P, H + 1], f32, tag="big")
                        nc.tensor.transpose(
                            ot,
                            o_sb[:, jj * P : (jj + 1) * P],
                            ident[0 : H + 1, 0 : H + 1],
                        )
                        recip = osbp.tile([P, 1], f32, tag="recip")
                        nc.vector.reciprocal(recip, ot[:, H : H + 1])
                        nc.vector.tensor_scalar_mul(o4[:, jj, :], ot[:, 0:H], recip)
                    q0 = qb * NB
                    nc.sync.dma_start(
                        out[q0 : q0 + NB, :].rearrange("(j p) h -> p j h", p=P), o4
                    )

    nc.compile()
    return nc


def _get_nc():
    if "nc" not in _cache:
        _cache["nc"] = _build(use_f32r=os.environ.get("K_NO_F32R", "") != "1")
    return _cache["nc"]


def _prep_inputs(x, Wk, bk, Wq, bq, Wv, bv):
    x = np.asarray(x, np.float32)
    wqt = np.asarray(Wq, np.float32).T
    wkt = np.asarray(Wk, np.float32).T
    wvt = np.ascontiguousarray(np.asarray(Wv, np.float32).T)
    wkq = np.ascontiguousarray(np.concatenate([wkt, wqt], axis=1))
    wvk = np.ascontiguousarray(np.concatenate([wvt, wkt], axis=1))
    bqc = np.asarray(bq, np.float32).reshape(H, 1)
    bkc = np.asarray(bk, np.float32).reshape(H, 1)
    bvc = np.asarray(bv, np.float32).reshape(H, 1)
    bkq = np.concatenate([bkc, bqc], axis=0)
    bvk = np.concatenate([bvc, bkc], axis=0)
    in_maps = []
    for i in range(NCORES):
        b, h = i // 2, i % 2
        xa = np.ascontiguousarray(x[b, h * TQ : (h + 1) * TQ])
        xbo = np.ascontiguousarray(x[b, (1 - h) * TQ : (2 - h) * TQ])
        in_maps.append(
            dict(xa=xa, xb=xbo, wkq=wkq, wvk=wvk, wvt=wvt,
                 bkq=bkq, bvk=bvk, bv=bvc)
        )
    return in_maps


def run(inputs, trace=False):
    from concourse.bass_utils import run_bass_kernel_spmd

    if not trace:
        # NTFF profiling is unavailable in this environment; make sure an
        # ambient BASS_TRACE can't divert the execute path.
        os.environ["BASS_NEVER_TRACE"] = "1"
    nc = _get_nc()
    in_maps = _prep_inputs(**inputs)
    res = run_bass_kernel_spmd(nc, in_maps, list(range(NCORES)), trace=trace)
    full = np.empty((B, T, H), np.float32)
    for i in range(NCORES):
        b, h = i // 2, i % 2
        full[b, h * TQ : (h + 1) * TQ] = res.results[i]["o"]
    return full, res


def kernel(**inputs):
    out, _ = run(inputs, trace=False)
    return out



# revision 9
# speedup vs baseline: 1.3795x; 1.3795x over previous
"""Trainium2 Bass kernel for nn_AttentionHead (B=4, T=4096, D=1024, H=64).

Sharding: 8 cores; core i handles (batch b = i//2, T-half = i%2): attention
output for its 2048 queries over all 4096 keys. K/V are computed per-core
(weights tiny/replicated; key order is permutation-invariant under softmax).

Design notes (cost-model-driven):
  - Everything bf16 on the PE (1 cyc/row at any moving size; fp32r would be
    4 cyc/row for the N=65 attnV matmuls). Host converts x/weights to bf16.
  - x^T comes straight from HBM via DMA-transpose (XBAR, 16x128 tiles,
    14 ns/tile): out[p, c, t] = x[t, c*128 + p], i.e. chunk c holds x^T rows
    c*128..c*128+127. No PE transposes, no PSUM->SBUF copies for x^T.
  - Projections W-stationary with M=128 packing: [Wk|Wq] on the own half,
    [Wk|Wv] on the other half; relu+bias on ACT evicts PSUM->SBUF bf16.
    kT lives in rows 0:64 of those tiles, qT in rows 64:128 (own), vT in
    rows 64:128 (other; PE-transposed back to natural V, 64 cyc/tile).
  - Own-half V x-stationary (stationary = x^T chunk [128d x 128t], moving
    Wv [128, 64]): N=64 per instr with all 128 PE rows used -> half the
    cost of a W-stationary V pass. Bias via 1-partition ones-row matmul
    into PSUM; relu on GPSIMD (Pool).
  - Scores per (q-block 512, k-tile 128): stationary kT [64,128], moving
    qT [64,512] -> s2 PSUM [128k, 512q] f32, two k-tiles per PSUM pair.
  - exp split across ACT (activation Exp, scale=1/8) and DVE (Schraudolph
    in bf16: i16 = round(s*(2^7/ln2)/8 + 127*128-6), bitcast to bf16;
    max rel err ~3.5%, averages out under softmax: end-to-end ~2e-3).
  - attnV flipped: stationary e-slice [128k, 128q], moving V [128, 65]
    (col 64 = ones -> denominator accumulates in o_ps col 64). Output is
    natural [q, h]; per q-tile: DVE reciprocal + Pool scale, DMA out f32.
"""

import os
import numpy as np
import ml_dtypes

B, T, D, H = 4, 4096, 1024, 64
P = 128
TQ = T // 2          # queries per core
NG = 8               # k-groups of 512 (4 own + 4 other)
NGH = 4
NQB = 4              # q-blocks of 512
NC = D // P          # 8 d-chunks
NCORES = 8

_cache = {}

EXP_A8 = float(2**7 / np.log(2) / 8.0)   # Schraudolph slope (scale 1/8 folded)
EXP_B = float(127 * 128 - 6)             # Schraudolph offset (tuned C=-6)


def _build():
    import concourse.bass as bass
    import concourse.tile as tile
    from concourse import bacc, mybir
    from concourse.masks import make_identity

    f32 = mybir.dt.float32
    bf16 = mybir.dt.bfloat16
    i16 = mybir.dt.int16
    AF = mybir.ActivationFunctionType
    ALU = mybir.AluOpType

    nc = bacc.Bacc("TRN2", target_bir_lowering=False, debug=False)

    xa = nc.dram_tensor("xa", [TQ, D], bf16, kind="ExternalInput").ap()
    xb = nc.dram_tensor("xb", [TQ, D], bf16, kind="ExternalInput").ap()
    wkq = nc.dram_tensor("wkq", [P, NC, P], bf16, kind="ExternalInput").ap()
    wkv = nc.dram_tensor("wkv", [P, NC, P], bf16, kind="ExternalInput").ap()
    wv = nc.dram_tensor("wv", [P, NC, H], bf16, kind="ExternalInput").ap()
    bkq = nc.dram_tensor("bkq", [P, 1], f32, kind="ExternalInput").ap()
    bkv = nc.dram_tensor("bkv", [P, 1], f32, kind="ExternalInput").ap()
    bvr = nc.dram_tensor("bvr", [1, H], bf16, kind="ExternalInput").ap()
    out = nc.dram_tensor("o", [TQ, H], f32, kind="ExternalOutput").ap()

    with tile.TileContext(nc) as tc:
        with (
            tc.tile_pool(name="const", bufs=1) as constp,
            tc.tile_pool(name="kq", bufs=1) as kqp,
            tc.tile_pool(name="kv", bufs=1) as kvp,
            tc.tile_pool(name="vt", bufs=1) as vp,
            tc.tile_pool(name="xt", bufs=2) as xtp,
            tc.tile_pool(name="proj_ps", bufs=1, space="PSUM") as projp,
            tc.tile_pool(name="s2_ps", bufs=2, space="PSUM") as s2p,
            tc.tile_pool(name="o_ps", bufs=2, space="PSUM") as op,
            tc.tile_pool(name="e", bufs=4) as ep,
            tc.tile_pool(name="outp", bufs=4) as outp,
        ):
            # ---- constants / persistent ----
            ident = constp.tile([P, P], f32)
            make_identity(nc, ident)
            ident_bf = constp.tile([P, P], bf16)
            nc.vector.tensor_copy(ident_bf, ident)
            onesr = constp.tile([1, P], bf16)
            nc.gpsimd.memset(onesr, 1.0)

            wkq_sb = constp.tile([P, NC, P], bf16)
            wkv_sb = constp.tile([P, NC, P], bf16)
            wv_sb = constp.tile([P, NC, H], bf16)
            bkq_sb = constp.tile([P, 1], f32)
            bkv_sb = constp.tile([P, 1], f32)
            bvr_sb = constp.tile([1, H], bf16)

            def load_consts():
                nc.sync.dma_start(wkq_sb, wkq)
                nc.sync.dma_start(wkv_sb, wkv)
                nc.sync.dma_start(wv_sb, wv)
                nc.sync.dma_start(bkq_sb, bkq)
                nc.sync.dma_start(bkv_sb, bkv)
                nc.sync.dma_start(bvr_sb, bvr)

            # persistent attention operands
            kq_sb = [kqp.tile([P, 512], bf16, name=f"kq{g}") for g in range(NGH)]
            kv_sb = [kvp.tile([P, 512], bf16, name=f"kv{g}") for g in range(NGH)]
            qt_sb = [kqp.tile([H, 512], bf16, name=f"qt{g}") for g in range(NGH)]
            V = vp.tile([P, T // P, H + 1], bf16)
            nc.gpsimd.memset(V[:, :, H : H + 1], 1.0)

            # ---- projections (per k-group of 512 t) ----
            def proj(g, after_dma=None):
                own = g < NGH
                src = xa if own else xb
                r0 = (g % NGH) * 512
                xt = xtp.tile([P, NC, 512], bf16, tag="xt")
                nc.sync.dma_start_transpose(xt, src[r0 : r0 + 512, :])
                if after_dma is not None:
                    after_dma()

                w2 = wkq_sb if own else wkv_sb
                b2 = bkq_sb if own else bkv_sb
                kq_ps = projp.tile([P, 512], f32, tag="proj", name="kq_ps")
                for c in range(NC):
                    nc.tensor.matmul(
                        kq_ps, w2[:, c, :], xt[:, c, :],
                        start=(c == 0), stop=(c == NC - 1),
                    )
                dst = kq_sb[g] if own else kv_sb[g - NGH]
                nc.scalar.activation(dst, kq_ps, AF.Relu, bias=b2[:, 0:1])
                if own:
                    # qT to a base-0 tile (scores operands must share base);
                    # SBUF->SBUF, so GPSIMD can take it (PSUM is off-limits)
                    nc.gpsimd.tensor_copy(qt_sb[g], dst[H:P, :])

                if own:
                    # natural-orientation V: stationary x^T chunks, M=128
                    v_ps = projp.tile([P, 4, H], f32, tag="proj", name="v_ps")
                    for i in range(4):
                        nc.tensor.matmul(
                            v_ps[:, i, :], onesr, bvr_sb, start=True, stop=False
                        )
                        for c in range(NC):
                            nc.tensor.matmul(
                                v_ps[:, i, :],
                                xt[:, c, i * P : (i + 1) * P],
                                wv_sb[:, c, :],
                                start=False, stop=(c == NC - 1),
                            )
                    nc.scalar.activation(
                        V[:, g * 4 : (g + 1) * 4, 0:H], v_ps, AF.Relu
                    )
                else:
                    # vT rows 64:128 of dst -> natural V via PE transpose
                    vt_ps = projp.tile([P, 4, H], bf16, tag="proj", name="vt_ps")
                    for i in range(4):
                        nc.tensor.transpose(
                            vt_ps[:, i, :],
                            dst[H:P, i * P : (i + 1) * P],
                            ident_bf[H:P, H:P],
                        )
                    nc.vector.tensor_copy(V[:, g * 4 : (g + 1) * 4, 0:H], vt_ps)

            for g in range(NG):
                proj(g, after_dma=load_consts if g == 0 else None)

            # ---- attention: q-blocks outer, k-groups inner ----
            n_exp = NQB * NG * 2
            n_act = 33  # ACT/DVE split balances engine time
            exp_idx = 0

            for qb in range(NQB):
                qT = qt_sb[qb]
                o_ps = op.tile([P, 4, H + 1], f32, tag="o", name=f"o_ps{qb}")
                for g in range(NG):
                    ksrc = kq_sb[g] if g < NGH else kv_sb[g - NGH]
                    for pair in range(2):
                        s2 = s2p.tile([P, 1024], f32, tag="s2")
                        for jj in range(2):
                            j = pair * 2 + jj
                            nc.tensor.matmul(
                                s2[:, jj * 512 : (jj + 1) * 512],
                                ksrc[0:H, j * P : (j + 1) * P],
                                qT,
                                start=True, stop=True,
                            )
                        use_act = (exp_idx * n_act) // n_exp != (
                            (exp_idx + 1) * n_act
                        ) // n_exp
                        exp_idx += 1
                        if use_act:
                            e = ep.tile([P, 1024], bf16, tag="e", name="e_act")
                            nc.scalar.activation(e, s2, AF.Exp, scale=0.125)
                        else:
                            ei = ep.tile([P, 1024], i16, tag="e", name="e_dve")
                            nc.vector.tensor_scalar(
                                ei, s2, EXP_A8, EXP_B, ALU.mult, ALU.add
                            )
                            e = ei.bitcast(bf16)
                        for jj in range(2):
                            j = pair * 2 + jj
                            kt = g * 4 + j
                            for i in range(4):
                                nc.tensor.matmul(
                                    o_ps[:, i, :],
                                    e[:, jj * 512 + i * P : jj * 512 + (i + 1) * P],
                                    V[:, kt, :],
                                    start=(kt == 0), stop=(kt == T // P - 1),
                                )
                # normalize + store (natural [q, h]; denominator in col 64)
                for i in range(4):
                    recip = outp.tile([P, 1], f32, tag="rcp")
                    nc.vector.reciprocal(recip, o_ps[:, i, H : H + 1])
                    osb = outp.tile([P, H], f32, tag="osb")
                    nc.vector.tensor_scalar_mul(osb, o_ps[:, i, 0:H], recip)
                    q0 = qb * 512 + i * P
                    nc.sync.dma_start(out[q0 : q0 + P, :], osb)

    nc.compile()
    return nc


def _get_nc():
    if "nc" not in _cache:
        _cache["nc"] = _build()
    return _cache["nc"]


def _prep_inputs(x, Wk, bk, Wq, bq, Wv, bv):
    bf = ml_dtypes.bfloat16
    x = np.asarray(x, np.float32)

    def chunked(wT):  # [D, M] -> [P, NC, M] with row d = c*128+p at [p, c]
        m = wT.shape[1]
        return np.ascontiguousarray(
            wT.reshape(NC, P, m).transpose(1, 0, 2).astype(bf)
        )

    wkt = np.asarray(Wk, np.float32).T
    wqt = np.asarray(Wq, np.float32).T
    wvt = np.asarray(Wv, np.float32).T
    wkq = chunked(np.concatenate([wkt, wqt], axis=1))
    wkv = chunked(np.concatenate([wkt, wvt], axis=1))
    wvc = chunked(wvt)
    bkc = np.asarray(bk, np.float32).reshape(H, 1)
    bqc = np.asarray(bq, np.float32).reshape(H, 1)
    bvc = np.asarray(bv, np.float32).reshape(H, 1)
    bkq = np.concatenate([bkc, bqc], axis=0)
    bkv = np.concatenate([bkc, bvc], axis=0)
    bvr = np.ascontiguousarray(bvc.reshape(1, H).astype(bf))

    in_maps = []
    for i in range(NCORES):
        b, h = i // 2, i % 2
        xa = np.ascontiguousarray(x[b, h * TQ : (h + 1) * TQ].astype(bf))
        xbo = np.ascontiguousarray(x[b, (1 - h) * TQ : (2 - h) * TQ].astype(bf))
        in_maps.append(
            dict(xa=xa, xb=xbo, wkq=wkq, wkv=wkv, wv=wvc,
                 bkq=bkq, bkv=bkv, bvr=bvr)
        )
    return in_maps


def run(inputs, trace=False):
    from concourse.bass_utils import run_bass_kernel_spmd

    if not trace:
        os.environ["BASS_NEVER_TRACE"] = "1"
    nc = _get_nc()
    in_maps = _prep_inputs(**inputs)
    res = run_bass_kernel_spmd(nc, in_maps, list(range(NCORES)), trace=trace)
    full = np.empty((B, T, H), np.float32)
    for i in range(NCORES):
        b, h = i // 2, i % 2
        full[b, h * TQ : (h + 1) * TQ] = res.results[i]["o"]
    return full, res


def kernel(**inputs):
    out, _ = run(inputs, trace=False)
    return out
